# revision 10
# baseline (speedup 1.0000x reference)
"""Distributed Trainium2 Bass kernel for causal multi-head attention w/ RoPE.

Problem shapes (hardcoded): B=2, S=2048, D=1024, H=16, HD=64.
Sharding: tensor-parallel over heads — each of 8 cores owns 2 heads
(column slice of wq/wk/wv, row slice of wo). Each core emits its partial
x @ woT contribution; the host sums the 8 partials (the "all-reduce").

Per-core math (all matmuls bf16 on the PE, fp32 PSUM accumulation):
  - q,k,v projections from a host-transposed xT [D, B*S].
  - RoPE: head dims are host-permuted to rotate-half form, so
    rot(q) = q*cos + (P@q)*sin with P a signed block-swap applied by one
    PE matmul; DVE elementwise ops finish the rotation.
  - scores computed transposed: sT[sk, sq] = k_h^T q_h (K=64), causal
    tiles only; exp on ScalarE with the 1/sqrt(HD) scale folded in.
  - the causal mask is applied ON the PE (identity @ maskT accumulated
    into the scores PSUM); one wide exp instruction per sk-tile.
  - the attention inner loop is software-pipelined two sk-tiles deep
    (emit scores of kt+1, exp of kt, PV of kt-1) so the PE never waits
    on the ScalarE exp and keeps its p-state ramp.
  - PV uses v' = [v | 1] so the softmax denominator falls out of the
    matmul as row 64 of the accumulator; normalization = DVE copy
    (frees PSUM early) + fast DVE reciprocal + gpsimd
    partition-broadcast + DVE multiply, off the PE critical path.
  - wo partials are emitted per 1024-column group as soon as that
    group is normalized, so the final-batch tail is short.
"""

import sys

sys.path.insert(0, "/opt/trn_rl_repo")

import numpy as np
import ml_dtypes

B, S, D, H = 2, 2048, 1024, 16
HD = D // H  # 64
NC = 8
HPC = H // NC  # heads per core = 2
HDC = HPC * HD  # head dims per core = 128
TOK = B * S  # 4096
BF16 = ml_dtypes.bfloat16

_COMPILED = {}


def _build_program():
    import concourse.bass as bass
    import concourse.mybir as mybir
    import concourse.bacc as bacc
    from concourse import tile

    f32 = mybir.dt.float32
    bf16 = mybir.dt.bfloat16
    MULT = mybir.AluOpType.mult
    ADD = mybir.AluOpType.add
    EXP = mybir.ActivationFunctionType.Exp

    nc = bacc.Bacc("TRN2", target_bir_lowering=False, debug=False, num_devices=NC)

    xT_d = nc.dram_tensor("xT", [(TOK // 512) * (D // 128) * 128, 512], bf16,
                          kind="ExternalInput").ap()
    wqT_d = nc.dram_tensor("wqT", [D, HDC], bf16, kind="ExternalInput").ap()
    wkT_d = nc.dram_tensor("wkT", [D, HDC], bf16, kind="ExternalInput").ap()
    wvT_d = nc.dram_tensor("wvT", [D, HDC], bf16, kind="ExternalInput").ap()
    woT_d = nc.dram_tensor("woT", [HDC, D], bf16, kind="ExternalInput").ap()
    PT_d = nc.dram_tensor("PT", [HDC, HDC], bf16, kind="ExternalInput").ap()
    cos_d = nc.dram_tensor("cosx", [HDC, TOK], bf16, kind="ExternalInput").ap()
    sin_d = nc.dram_tensor("sinx", [HDC, TOK], bf16, kind="ExternalInput").ap()
    tri_d = nc.dram_tensor("tri", [128, 128], bf16, kind="ExternalInput").ap()
    id_d = nc.dram_tensor("ident", [128, 128], bf16, kind="ExternalInput").ap()
    one_d = nc.dram_tensor("ones", [1, 128], bf16, kind="ExternalInput").ap()
    out_d = nc.dram_tensor("out", [D, TOK], bf16, kind="ExternalOutput").ap()

    KT = D // 128  # 8 contraction tiles for projections
    NTB = TOK // 512  # 8 tok blocks of 512
    NVT = TOK // 128  # 32 tok tiles of 128 (v' tiles)
    VW = HD + 1  # 65: v plus ones column

    with tile.TileContext(nc) as tc:
        with (
            tc.tile_pool(name="big", bufs=1) as big,
            tc.tile_pool(name="work", bufs=3) as work,
            tc.tile_pool(name="etp", bufs=3) as etp,
            tc.tile_pool(name="nrm", bufs=3) as nrm,
            tc.tile_pool(name="scps", bufs=3, space="PSUM") as scps,
            tc.tile_pool(name="pop", bufs=2, space="PSUM") as pop,
        ):
            # ---- DMA order: first-needed tiles first --------------------
            wq = big.tile([128, KT * HDC], bf16, tag="wq")
            wk = big.tile([128, KT * HDC], bf16, tag="wk")
            wv = big.tile([128, KT * HDC], bf16, tag="wv")
            xT = big.tile([128, KT * TOK], bf16, tag="xT")

            dmaq = [nc.sync, nc.gpsimd, nc.scalar]

            def xт_chunk_dma(tb, k, eng):
                r0 = (tb * KT + k) * 128
                eng.dma_start(
                    xT[:, k * TOK + tb * 512 : k * TOK + (tb + 1) * 512],
                    xT_d[r0 : r0 + 128, :])

            for k in range(KT):  # first projection tile's needs
                nc.sync.dma_start(wq[:, k * HDC : (k + 1) * HDC],
                                  wqT_d[k * 128 : (k + 1) * 128, :])
                xт_chunk_dma(0, k, dmaq[1 + k % 2])
            for w_sb, w_d in ((wk, wkT_d), (wv, wvT_d)):
                for k in range(KT):
                    nc.sync.dma_start(w_sb[:, k * HDC : (k + 1) * HDC],
                                      w_d[k * 128 : (k + 1) * 128, :])
            for tb in range(1, NTB):
                for k in range(KT):
                    xт_chunk_dma(tb, k, dmaq[k % 3])
            wo = big.tile([128, D], bf16, tag="wo")
            nc.sync.dma_start(wo[:], woT_d[:, :])
            PT = big.tile([128, 128], bf16, tag="PT")
            nc.sync.dma_start(PT[:], PT_d[:, :])
            cosx = big.tile([128, TOK], bf16, tag="cosx")
            nc.gpsimd.dma_start(cosx[:], cos_d[:, :])
            sinx = big.tile([128, TOK], bf16, tag="sinx")
            nc.scalar.dma_start(sinx[:], sin_d[:, :])
            tri = big.tile([128, 128], bf16, tag="tri")
            nc.sync.dma_start(tri[:], tri_d[:, :])
            ident = big.tile([128, 128], bf16, tag="ident")
            nc.sync.dma_start(ident[:], id_d[:, :])

            q_sb = big.tile([128, TOK], bf16, tag="q")
            k_sb = big.tile([128, TOK], bf16, tag="k")
            v_sb = big.tile([128, TOK], bf16, tag="v")
            rotq = big.tile([128, TOK], bf16, tag="rotq")
            rotk = big.tile([128, TOK], bf16, tag="rotk")
            vp = big.tile([128, NVT * 2 * VW], bf16, tag="vp")
            oh1 = big.tile([64, S], bf16, tag="oh1")

            def vp_head(kt, h):
                base = kt * 2 * VW + h * VW
                return vp[:, base : base + VW]

            def setup_batch(b):
                """projections + RoPE + v' tiles for batch b's 2048 cols"""
                for dst, w_sb in ((q_sb, wq), (k_sb, wk), (v_sb, wv)):
                    for tb in range(b * 4, b * 4 + 4):
                        ps = scps.tile([128, 512], f32, tag="sc",
                                       name=f"ps{b}{tb}")
                        for k in range(KT):
                            nc.tensor.matmul(
                                ps[:],
                                w_sb[:, k * HDC : (k + 1) * HDC],
                                xT[:, k * TOK + tb * 512 : k * TOK + (tb + 1) * 512],
                                start=(k == 0), stop=(k == KT - 1),
                            )
                        if b == 0:
                            nc.scalar.copy(dst[:, tb * 512 : (tb + 1) * 512],
                                           ps[:])
                        else:
                            nc.vector.tensor_copy(
                                dst[:, tb * 512 : (tb + 1) * 512], ps[:])
                for src, dst in ((q_sb, rotq), (k_sb, rotk)):
                    for tb in range(b * 4, b * 4 + 4):
                        blk = slice(tb * 512, (tb + 1) * 512)
                        pss = scps.tile([128, 512], f32, tag="sc",
                                        name=f"pr{b}{tb}")
                        nc.tensor.matmul(pss[:], PT[:], src[:, blk],
                                         start=True, stop=True)
                        t1 = work.tile([128, 512], bf16, tag="ropet1")
                        nc.vector.tensor_tensor(t1[:], src[:, blk],
                                                cosx[:, blk], MULT)
                        t2 = work.tile([128, 512], bf16, tag="ropet2")
                        nc.vector.tensor_tensor(t2[:], pss[:], sinx[:, blk], MULT)
                        nc.vector.tensor_tensor(dst[:, blk], t1[:], t2[:], ADD)
                for kt in range(b * 16, b * 16 + 16):
                    pst = scps.tile([128, 128], bf16, tag="sc",
                                    name=f"pv{kt}")
                    nc.tensor.transpose(pst[:],
                                        v_sb[:, kt * 128 : (kt + 1) * 128],
                                        ident[:])
                    for h in range(HPC):
                        base = kt * 2 * VW + h * VW
                        nc.vector.tensor_copy(vp[:, base : base + HD],
                                              pst[:, h * HD : (h + 1) * HD])
                        nc.gpsimd.memset(vp[:, base + HD : base + VW], 1.0)

            def attention_group(b, h, g, outT_b):
                """scores+softmax+PV for head h, sq cols [g*1024,(g+1)*1024)

                Two-deep software pipeline: iteration kt emits scores of
                kt+1, the exp of kt, and PV of kt-1, so the PE is never
                blocked on the ScalarE exp."""
                hsl = slice(h * HD, (h + 1) * HD)
                g0 = 1024 * g
                n_kt = 8 * g + 8
                kt_max = [(g0 + 512 * (i + 1)) // 128 - 1 for i in range(2)]
                po = [pop.tile([VW, 512], f32, tag="po",
                               name=f"po{b}{h}{g}{i}") for i in range(2)]
                sc = {}
                et = {}

                def cols(kt, i):
                    w0 = 128 * kt
                    s0 = max(w0, g0 + 512 * i)
                    e0 = g0 + 512 * (i + 1)
                    return (s0, e0) if s0 < e0 else None

                def emit_S(kt):
                    w0 = 128 * kt
                    t = scps.tile([128, 1024], f32, tag="sc",
                                  name=f"sc{b}{h}{g}{kt}")
                    sc[kt] = t
                    diag_i = (w0 - g0) // 512 if w0 >= g0 else -1
                    for i in range(2):
                        ci = cols(kt, i)
                        if ci is None:
                            continue
                        s0, e0 = ci
                        nc.tensor.matmul(
                            t[:, s0 - g0 : e0 - g0],
                            rotk[hsl, b * S + w0 : b * S + w0 + 128],
                            rotq[hsl, b * S + s0 : b * S + e0],
                            start=True, stop=(i != diag_i),
                        )
                    if diag_i >= 0:  # diagonal tile: accumulate -1e9 mask
                        nc.tensor.matmul(
                            t[:, w0 - g0 : w0 - g0 + 128],
                            ident[:], tri[:],
                            start=False, stop=True,
                        )

                def emit_E(kt):
                    c0 = max(128 * kt, g0)
                    tt = etp.tile([128, 1024], bf16, tag="expT",
                                  name=f"et{b}{h}{g}{kt}")
                    et[kt] = tt
                    nc.scalar.activation(
                        tt[:, c0 - g0 : 1024],
                        sc[kt][:, c0 - g0 : 1024],
                        EXP, scale=0.125)
                    del sc[kt]

                def emit_PV(kt):
                    for i in range(2):
                        ci = cols(kt, i)
                        if ci is None:
                            continue
                        s0, e0 = ci
                        h0c = g0 + 512 * i
                        nc.tensor.matmul(
                            po[i][:, s0 - h0c : e0 - h0c],
                            vp_head(b * (NVT // B) + kt, h),
                            et[kt][:, s0 - g0 : e0 - g0],
                            start=(kt == 0), stop=(kt == kt_max[i]),
                        )
                    if kt == kt_max[0]:
                        norm_half(0)

                def norm_half(i):
                    c0 = g0 + 512 * i
                    poc = nrm.tile([VW, 512], f32, tag="poc")
                    nc.vector.tensor_copy(poc[:], po[i][:])  # frees PSUM slot
                    d0 = nrm.tile([1, 512], f32, tag="d0")
                    nc.gpsimd.tensor_copy(d0[0:1, :], poc[HD : HD + 1, :])
                    db = nrm.tile([64, 512], f32, tag="db")
                    nc.gpsimd.partition_broadcast(db[:, :], d0[0:1, :])
                    rb_sb = nrm.tile([64, 512], f32, tag="rb")
                    nc.vector.reciprocal_approx_fast(rb_sb[:, :], db[:, :])
                    ocols = slice(c0, c0 + 512)
                    if h == 0:
                        nc.vector.tensor_tensor(outT_b[0:HD, ocols],
                                                poc[0:HD, :], rb_sb[:, :], MULT)
                    else:
                        nc.vector.tensor_tensor(oh1[:, ocols], poc[0:HD, :],
                                                rb_sb[:, :], MULT)
                        nc.sync.dma_start(outT_b[HD : 2 * HD, ocols],
                                          oh1[:, ocols])

                emit_S(0)
                if n_kt > 1:
                    emit_S(1)
                emit_E(0)
                for kt in range(1, n_kt):
                    if kt + 1 < n_kt:
                        emit_S(kt + 1)
                    emit_E(kt)
                    emit_PV(kt - 1)
                emit_PV(n_kt - 1)
                norm_half(1)

            def wo_group(b, g, outT_b):
                """wo partial for outT cols [g*1024,(g+1)*1024)"""
                for o in range(D // 128):
                    for ti, t4 in enumerate((2 * g, 2 * g + 1)):
                        psw = scps.tile([128, 512], f32, tag="sc",
                                        name=f"pw{b}{o}{t4}")
                        nc.tensor.matmul(
                            psw[:],
                            wo[:, o * 128 : (o + 1) * 128],
                            outT_b[:, t4 * 512 : (t4 + 1) * 512],
                            start=True, stop=True,
                        )
                        wout = work.tile([128, 512], bf16, tag="wout")
                        if (o + ti) % 2 == 0:
                            nc.vector.tensor_copy(wout[:], psw[:])
                        else:
                            nc.scalar.copy(wout[:], psw[:])
                        dmaq[(o + 2 * ti) % 2].dma_start(
                            out_d[o * 128 : (o + 1) * 128,
                                  b * S + t4 * 512 : b * S + (t4 + 1) * 512],
                            wout[:])

            setup_batch(0)
            outT_b0 = big.tile([128, S], bf16, tag="outT0")
            attention_group(0, 1, 0, outT_b0)
            attention_group(0, 0, 0, outT_b0)
            attention_group(0, 1, 1, outT_b0)
            wo_group(0, 0, outT_b0)
            attention_group(0, 0, 1, outT_b0)
            setup_batch(1)   # fills PE time while batch-0 tail drains
            wo_group(0, 1, outT_b0)
            outT_b1 = big.tile([128, S], bf16, tag="outT1")
            attention_group(1, 1, 0, outT_b1)
            attention_group(1, 0, 0, outT_b1)
            attention_group(1, 1, 1, outT_b1)
            wo_group(1, 0, outT_b1)
            attention_group(1, 0, 1, outT_b1)
            wo_group(1, 1, outT_b1)

    nc.compile()
    return nc


def _host_inputs(x, wq, wk, wv, wo, freqs_cos, freqs_sin):
    """Build the per-core input maps (all host-side transforms are free)."""
    perm = np.concatenate([np.arange(0, HD, 2), np.arange(1, HD, 2)])  # rot-half
    xTf = x.reshape(TOK, D).T.astype(BF16)  # [D, TOK]
    # chunk-contiguous tiling: row block (tb*KT+k) holds xT[k*128:+128, tb*512:+512]
    xT = np.zeros(((TOK // 512) * (D // 128) * 128, 512), BF16)
    for tb in range(TOK // 512):
        for k in range(D // 128):
            r0 = (tb * (D // 128) + k) * 128
            xT[r0 : r0 + 128, :] = xTf[k * 128 : (k + 1) * 128,
                                       tb * 512 : (tb + 1) * 512]

    # signed block-swap P (per 64-dim head): qs_lo = -q_hi, qs_hi = q_lo
    P = np.zeros((HDC, HDC), np.float32)
    for h in range(HPC):
        base = h * HD
        half = HD // 2
        for i in range(half):
            P[base + i, base + half + i] = -1.0
            P[base + half + i, base + i] = 1.0
    PT = np.ascontiguousarray(P.T).astype(BF16)

    # cos/sin expanded to [HDC, TOK]; row j within a head uses freq j%32
    half = HD // 2
    idx = np.concatenate([np.arange(half), np.arange(half)])  # [64]
    cos1 = freqs_cos[:, :].T[idx]  # [64, S]
    sin1 = freqs_sin[:, :].T[idx]
    cosx = np.tile(np.tile(cos1, (HPC, 1)), (1, B)).astype(BF16)  # [128, TOK]
    sinx = np.tile(np.tile(sin1, (HPC, 1)), (1, B)).astype(BF16)

    # additive causal mask for the diagonal tile: 0 where sk<=sq, -1e9 else
    tri = np.where(np.triu(np.ones((128, 128), dtype=bool)), 0.0,
                   -1e9).astype(BF16)
    ident = np.eye(128, dtype=np.float32).astype(BF16)
    ones = np.ones((1, 128), np.float32).astype(BF16)

    in_maps = []
    for c in range(NC):
        rows = []
        for h in range(HPC):
            hg = c * HPC + h
            rows.append(hg * HD + perm)
        rows = np.concatenate(rows)
        wq_c = np.ascontiguousarray(wq[rows, :].T).astype(BF16)  # [D, 128]
        wk_c = np.ascontiguousarray(wk[rows, :].T).astype(BF16)
        vrows = np.arange(c * HDC, (c + 1) * HDC)
        wv_c = np.ascontiguousarray(wv[vrows, :].T).astype(BF16)
        wo_c = np.ascontiguousarray(wo[:, vrows].T).astype(BF16)  # [128, D]
        in_maps.append({
            "xT": xT, "wqT": wq_c, "wkT": wk_c, "wvT": wv_c, "woT": wo_c,
            "PT": PT, "cosx": cosx, "sinx": sinx, "tri": tri,
            "ident": ident, "ones": ones,
        })
    return in_maps


def _install_ntff_hook():
    """Provide antenv.axon_hooks (missing in this image) so that
    run_bass_kernel_spmd(trace=True) can capture an NTFF profile via the
    axon PJRT .so — replicates trn_boot._ntff_profile_via_ctypes."""
    import types, ctypes, contextlib, sys as _sys

    if "antenv.axon_hooks" in _sys.modules:
        return
    so_path = "/opt/axon/libaxon_pjrt.so"
    try:
        lib = ctypes.CDLL(so_path)
    except OSError:
        return
    if not hasattr(lib, "axon_start_nrt_profile"):
        return
    lib.axon_start_nrt_profile.argtypes = [ctypes.POINTER(ctypes.c_int64),
                                           ctypes.c_size_t]
    lib.axon_start_nrt_profile.restype = ctypes.c_int64
    lib.axon_stop_nrt_profile.argtypes = [ctypes.c_char_p]
    lib.axon_stop_nrt_profile.restype = ctypes.c_int64

    @contextlib.contextmanager
    def _hook(output_dir, device_ids):
        import jax
        jax.devices()
        if device_ids:
            ids = (ctypes.c_int64 * len(device_ids))(*device_ids)
            rc = lib.axon_start_nrt_profile(ids, len(device_ids))
        else:
            rc = lib.axon_start_nrt_profile(None, 0)
        if rc != 0:
            raise RuntimeError(f"axon_start_nrt_profile rc={rc}")
        try:
            yield
        finally:
            n = lib.axon_stop_nrt_profile(str(output_dir).encode())
            print(f"ntff profile: {n} file(s) -> {output_dir}", file=sys.stderr)

    mod = types.ModuleType("antenv.axon_hooks")
    mod.get_axon_ntff_profile_hook = lambda: _hook
    mod.set_axon_ntff_profile_hook = lambda h: None
    import antenv
    antenv.axon_hooks = mod
    _sys.modules["antenv.axon_hooks"] = mod


def _is_causal_mask(mask):
    ref = np.where(np.tril(np.ones((S, S), dtype=bool)), 0.0, -1e9)
    return mask.shape == (S, S) and np.array_equal(
        mask.astype(np.float32), ref.astype(np.float32))


def kernel(x, wq, wk, wv, wo, freqs_cos, freqs_sin, mask, _want_trace=False):
    x = np.asarray(x, np.float32)
    mask = np.asarray(mask, np.float32)
    if not _is_causal_mask(mask):
        # general fallback (never hit for the reference's causal mask)
        return _numpy_reference(x, wq, wk, wv, wo, freqs_cos, freqs_sin, mask)

    from concourse.bass_utils import run_bass_kernel_spmd

    if _want_trace:
        _install_ntff_hook()
    if "prog" not in _COMPILED:
        _COMPILED["prog"] = _build_program()
    nc = _COMPILED["prog"]

    in_maps = _host_inputs(np.asarray(x, np.float32), np.asarray(wq, np.float32),
                           np.asarray(wk, np.float32), np.asarray(wv, np.float32),
                           np.asarray(wo, np.float32),
                           np.asarray(freqs_cos, np.float32),
                           np.asarray(freqs_sin, np.float32))
    res = run_bass_kernel_spmd(nc, in_maps, core_ids=list(range(NC)),
                               trace=_want_trace)
    total = np.zeros((D, TOK), np.float32)
    for c in range(NC):
        total += res.results[c]["out"].astype(np.float32)
    out = total.T.reshape(B, S, D).astype(np.float32)
    if _want_trace:
        _COMPILED["last_result"] = res
    return out


def _numpy_reference(x, wq, wk, wv, wo, freqs_cos, freqs_sin, mask):
    import math

    def rope(t):
        t2 = t.reshape(*t.shape[:-1], HD // 2, 2)
        x0, x1 = t2[..., 0], t2[..., 1]
        c = freqs_cos[None, :, None, :]
        s = freqs_sin[None, :, None, :]
        r0 = x0 * c - x1 * s
        r1 = x0 * s + x1 * c
        return np.stack([r0, r1], axis=-1).reshape(t.shape)

    b, s, d = x.shape
    q = (x @ wq.T).reshape(b, s, H, HD)
    k = (x @ wk.T).reshape(b, s, H, HD)
    v = (x @ wv.T).reshape(b, s, H, HD)
    q, k = rope(q), rope(k)
    q = q.transpose(0, 2, 1, 3)
    k = k.transpose(0, 2, 1, 3)
    v = v.transpose(0, 2, 1, 3)
    sc = np.einsum("bhqd,bhkd->bhqk", q, k) / math.sqrt(HD) + mask[None, None]
    sc = sc - sc.max(axis=-1, keepdims=True)
    p = np.exp(sc)
    p /= p.sum(axis=-1, keepdims=True)
    o = np.einsum("bhqk,bhkd->bhqd", p, v).transpose(0, 2, 1, 3).reshape(b, s, d)
    return (o @ wo.T).astype(np.float32)


# revision 14
# speedup vs baseline: 1.1778x; 1.1778x over previous
"""Distributed Trainium2 Bass kernel for causal multi-head attention w/ RoPE.

Problem shapes (hardcoded): B=2, S=2048, D=1024, H=16, HD=64.
Sharding: tensor-parallel over heads — each of 8 cores owns 2 heads
(column slice of wq/wk/wv, row slice of wo). Each core emits its partial
x @ woT contribution; the host sums the 8 partials (the "all-reduce").

Per-core math (all matmuls bf16 on the PE, fp32 PSUM accumulation):
  - q,k,v projections from a host-transposed xT [D, B*S].
  - RoPE: head dims are host-permuted to rotate-half form, so
    rot(q) = q*cos + (P@q)*sin with P a signed block-swap applied by one
    PE matmul; DVE elementwise ops finish the rotation.
  - scores computed transposed: sT[sk, sq] = k_h^T q_h (K=64), causal
    tiles only; exp on ScalarE with the 1/sqrt(HD) scale folded in.
  - the causal mask is applied ON the PE (identity @ maskT accumulated
    into the scores PSUM); one wide exp instruction per sk-tile.
  - the attention inner loop is software-pipelined two sk-tiles deep
    (emit scores of kt+1, exp of kt, PV of kt-1) so the PE never waits
    on the ScalarE exp and keeps its p-state ramp.
  - PV uses v' = [v | 1] so the softmax denominator falls out of the
    matmul as row 64 of the accumulator; normalization = DVE copy
    (frees PSUM early) + fast DVE reciprocal + gpsimd
    partition-broadcast + DVE multiply, off the PE critical path.
  - wo partials are emitted per 1024-column group as soon as that
    group is normalized, so the final-batch tail is short.
"""

import sys

sys.path.insert(0, "/opt/trn_rl_repo")

import numpy as np
import ml_dtypes

B, S, D, H = 2, 2048, 1024, 16
HD = D // H  # 64
NC = 8
HPC = H // NC  # heads per core = 2
HDC = HPC * HD  # head dims per core = 128
TOK = B * S  # 4096
BF16 = ml_dtypes.bfloat16

_COMPILED = {}


def _build_program():
    import concourse.bass as bass
    import concourse.mybir as mybir
    import concourse.bacc as bacc
    from concourse import tile

    f32 = mybir.dt.float32
    bf16 = mybir.dt.bfloat16
    MULT = mybir.AluOpType.mult
    ADD = mybir.AluOpType.add
    EXP = mybir.ActivationFunctionType.Exp

    nc = bacc.Bacc("TRN2", target_bir_lowering=False, debug=False, num_devices=NC)

    xT_d = nc.dram_tensor("xT", [(TOK // 512) * (D // 128) * 128, 512], bf16,
                          kind="ExternalInput").ap()
    wqT_d = nc.dram_tensor("wqT", [D, HDC], bf16, kind="ExternalInput").ap()
    wkT_d = nc.dram_tensor("wkT", [D, HDC], bf16, kind="ExternalInput").ap()
    wvT_d = nc.dram_tensor("wvT", [D, HDC], bf16, kind="ExternalInput").ap()
    woT_d = nc.dram_tensor("woT", [HDC, D], bf16, kind="ExternalInput").ap()
    PT_d = nc.dram_tensor("PT", [HDC, HDC], bf16, kind="ExternalInput").ap()
    cos_d = nc.dram_tensor("cosx", [HDC, TOK], bf16, kind="ExternalInput").ap()
    sin_d = nc.dram_tensor("sinx", [HDC, TOK], bf16, kind="ExternalInput").ap()
    tri_d = nc.dram_tensor("tri", [128, 128], bf16, kind="ExternalInput").ap()
    id_d = nc.dram_tensor("ident", [128, 128], bf16, kind="ExternalInput").ap()
    one_d = nc.dram_tensor("ones", [1, 128], bf16, kind="ExternalInput").ap()
    out_d = nc.dram_tensor("out", [D, TOK], bf16, kind="ExternalOutput").ap()

    KT = D // 128  # 8 contraction tiles for projections
    NTB = TOK // 512  # 8 tok blocks of 512
    NVT = TOK // 128  # 32 tok tiles of 128 (v' tiles)
    VW = HD + 1  # 65: v plus ones column

    with tile.TileContext(nc) as tc:
        with (
            tc.tile_pool(name="big", bufs=1) as big,
            tc.tile_pool(name="work", bufs=3) as work,
            tc.tile_pool(name="etp", bufs=4) as etp,
            tc.tile_pool(name="nrm", bufs=3) as nrm,
            tc.tile_pool(name="scps", bufs=4, space="PSUM") as scps,
            tc.tile_pool(name="psp", bufs=2, space="PSUM") as psp,
            tc.tile_pool(name="pop", bufs=2, space="PSUM") as pop,
        ):
            # ---- DMA order: first-needed tiles first --------------------
            wq = big.tile([128, KT * HDC], bf16, tag="wq")
            wk = big.tile([128, KT * HDC], bf16, tag="wk")
            wv = big.tile([128, KT * HDC], bf16, tag="wv")
            xT = big.tile([128, KT * TOK], bf16, tag="xT")

            dmaq = [nc.sync, nc.gpsimd, nc.scalar]

            def xт_chunk_dma(tb, k, eng):
                r0 = (tb * KT + k) * 128
                eng.dma_start(
                    xT[:, k * TOK + tb * 512 : k * TOK + (tb + 1) * 512],
                    xT_d[r0 : r0 + 128, :])

            for k in range(KT):  # first projection tile's needs
                nc.sync.dma_start(wq[:, k * HDC : (k + 1) * HDC],
                                  wqT_d[k * 128 : (k + 1) * 128, :])
                xт_chunk_dma(0, k, dmaq[1 + k % 2])
            # batch-0 rope tables early: rope(b0) starts ~10us in
            PT = big.tile([128, 128], bf16, tag="PT")
            nc.sync.dma_start(PT[:], PT_d[:, :])
            cosx = big.tile([128, TOK], bf16, tag="cosx")
            sinx = big.tile([128, TOK], bf16, tag="sinx")
            nc.gpsimd.dma_start(cosx[:, 0:S], cos_d[:, 0:S])
            nc.scalar.dma_start(sinx[:, 0:S], sin_d[:, 0:S])
            for w_sb, w_d in ((wk, wkT_d), (wv, wvT_d)):
                for k in range(KT):
                    nc.sync.dma_start(w_sb[:, k * HDC : (k + 1) * HDC],
                                      w_d[k * 128 : (k + 1) * 128, :])
            tri = big.tile([128, 128], bf16, tag="tri")
            nc.sync.dma_start(tri[:], tri_d[:, :])
            ident = big.tile([128, 128], bf16, tag="ident")
            nc.sync.dma_start(ident[:], id_d[:, :])
            for tb in range(1, 4):  # rest of batch 0
                for k in range(KT):
                    xт_chunk_dma(tb, k, dmaq[k % 3])
            wo = big.tile([128, D], bf16, tag="wo")
            nc.sync.dma_start(wo[:], woT_d[:, :])
            for tb in range(4, NTB):  # batch 1 (needed ~60us in)
                for k in range(KT):
                    xт_chunk_dma(tb, k, dmaq[k % 3])
            nc.gpsimd.dma_start(cosx[:, S:TOK], cos_d[:, S:TOK])
            nc.scalar.dma_start(sinx[:, S:TOK], sin_d[:, S:TOK])

            q_sb = big.tile([128, TOK], bf16, tag="q")
            k_sb = big.tile([128, TOK], bf16, tag="k")
            v_sb = big.tile([128, TOK], bf16, tag="v")
            rotq = big.tile([128, TOK], bf16, tag="rotq")
            rotk = big.tile([128, TOK], bf16, tag="rotk")
            vp = big.tile([128, NVT * 2 * VW], bf16, tag="vp")
            oh1 = big.tile([64, S], bf16, tag="oh1")

            def vp_head(kt, h):
                base = kt * 2 * VW + h * VW
                return vp[:, base : base + VW]

            def setup_batch(b):
                """projections + RoPE + v' tiles for batch b's 2048 cols"""
                for dst, w_sb in ((q_sb, wq), (k_sb, wk), (v_sb, wv)):
                    for tb in range(b * 4, b * 4 + 4):
                        ps = psp.tile([128, 512], f32, tag="mm",
                                      name=f"ps{b}{tb}")
                        for k in range(KT):
                            nc.tensor.matmul(
                                ps[:],
                                w_sb[:, k * HDC : (k + 1) * HDC],
                                xT[:, k * TOK + tb * 512 : k * TOK + (tb + 1) * 512],
                                start=(k == 0), stop=(k == KT - 1),
                            )
                        if b == 0:
                            nc.scalar.copy(dst[:, tb * 512 : (tb + 1) * 512],
                                           ps[:])
                        else:
                            nc.vector.tensor_copy(
                                dst[:, tb * 512 : (tb + 1) * 512], ps[:])
                for src, dst in ((q_sb, rotq), (k_sb, rotk)):
                    for tb in range(b * 4, b * 4 + 4):
                        blk = slice(tb * 512, (tb + 1) * 512)
                        pss = psp.tile([128, 512], f32, tag="mm",
                                        name=f"pr{b}{tb}")
                        nc.tensor.matmul(pss[:], PT[:], src[:, blk],
                                         start=True, stop=True)
                        t1 = work.tile([128, 512], bf16, tag="ropet1")
                        nc.vector.tensor_tensor(t1[:], src[:, blk],
                                                cosx[:, blk], MULT)
                        t2 = work.tile([128, 512], bf16, tag="ropet2")
                        nc.vector.tensor_tensor(t2[:], pss[:], sinx[:, blk], MULT)
                        nc.vector.tensor_tensor(dst[:, blk], t1[:], t2[:], ADD)
                for kt in range(b * 16, b * 16 + 16):
                    pst = psp.tile([128, 128], bf16, tag="mm",
                                    name=f"pv{kt}")
                    nc.tensor.transpose(pst[:],
                                        v_sb[:, kt * 128 : (kt + 1) * 128],
                                        ident[:])
                    for h in range(HPC):
                        base = kt * 2 * VW + h * VW
                        nc.vector.tensor_copy(vp[:, base : base + HD],
                                              pst[:, h * HD : (h + 1) * HD])
                        nc.gpsimd.memset(vp[:, base + HD : base + VW], 1.0)

            def attention_group(b, h, g, outT_b):
                """scores+softmax+PV for head h, sq cols [g*1024,(g+1)*1024)

                Two-deep software pipeline: iteration kt emits scores of
                kt+1, the exp of kt, and PV of kt-1, so the PE is never
                blocked on the ScalarE exp."""
                hsl = slice(h * HD, (h + 1) * HD)
                g0 = 1024 * g
                n_kt = 8 * g + 8
                kt_max = [(g0 + 512 * (i + 1)) // 128 - 1 for i in range(2)]
                po = [pop.tile([VW, 512], f32, tag="po",
                               name=f"po{b}{h}{g}{i}") for i in range(2)]
                sc = {}
                et = {}

                def cols(kt, i):
                    w0 = 128 * kt
                    s0 = max(w0, g0 + 512 * i)
                    e0 = g0 + 512 * (i + 1)
                    return (s0, e0) if s0 < e0 else None

                def emit_S(kt):
                    w0 = 128 * kt
                    diag_i = (w0 - g0) // 512 if w0 >= g0 else -1
                    for i in range(2):
                        ci = cols(kt, i)
                        if ci is None:
                            continue
                        s0, e0 = ci
                        h0c = g0 + 512 * i
                        t = scps.tile([128, 512], f32, tag="sc",
                                      name=f"sc{b}{h}{g}{kt}{i}")
                        sc[(kt, i)] = t
                        nc.tensor.matmul(
                            t[:, s0 - h0c : e0 - h0c],
                            rotk[hsl, b * S + w0 : b * S + w0 + 128],
                            rotq[hsl, b * S + s0 : b * S + e0],
                            start=True, stop=(i != diag_i),
                        )
                        if i == diag_i:  # diag tile: accumulate -1e9 mask
                            nc.tensor.matmul(
                                t[:, w0 - h0c : w0 - h0c + 128],
                                ident[:], tri[:],
                                start=False, stop=True,
                            )

                def emit_E(kt):
                    for i in range(2):
                        ci = cols(kt, i)
                        if ci is None:
                            continue
                        s0, e0 = ci
                        h0c = g0 + 512 * i
                        tt = etp.tile([128, 512], bf16, tag="expT",
                                      name=f"et{b}{h}{g}{kt}{i}")
                        et[(kt, i)] = tt
                        nc.scalar.activation(
                            tt[:, s0 - h0c : e0 - h0c],
                            sc.pop((kt, i))[:, s0 - h0c : e0 - h0c],
                            EXP, scale=0.125)

                def emit_PV(kt):
                    for i in range(2):
                        ci = cols(kt, i)
                        if ci is None:
                            continue
                        s0, e0 = ci
                        h0c = g0 + 512 * i
                        nc.tensor.matmul(
                            po[i][:, s0 - h0c : e0 - h0c],
                            vp_head(b * (NVT // B) + kt, h),
                            et.pop((kt, i))[:, s0 - h0c : e0 - h0c],
                            start=(kt == 0), stop=(kt == kt_max[i]),
                        )
                    if kt == kt_max[0]:
                        norm_half(0)

                def norm_half(i):
                    c0 = g0 + 512 * i
                    poc = nrm.tile([VW, 512], f32, tag="poc")
                    nc.vector.tensor_copy(poc[:], po[i][:])  # frees PSUM slot
                    d0 = nrm.tile([1, 512], f32, tag="d0")
                    nc.vector.tensor_copy(d0[0:1, :], poc[HD : HD + 1, :])
                    db = nrm.tile([64, 512], f32, tag="db")
                    nc.gpsimd.partition_broadcast(db[:, :], d0[0:1, :])
                    rb_sb = nrm.tile([64, 512], f32, tag="rb")
                    nc.vector.reciprocal_approx_fast(rb_sb[:, :], db[:, :])
                    ocols = slice(c0, c0 + 512)
                    if h == 0:
                        nc.vector.tensor_tensor(outT_b[0:HD, ocols],
                                                poc[0:HD, :], rb_sb[:, :], MULT)
                    else:
                        nc.vector.tensor_tensor(oh1[:, ocols], poc[0:HD, :],
                                                rb_sb[:, :], MULT)
                        nc.sync.dma_start(outT_b[HD : 2 * HD, ocols],
                                          oh1[:, ocols])

                emit_S(0)
                if n_kt > 1:
                    emit_S(1)
                emit_E(0)
                for kt in range(1, n_kt):
                    if kt + 1 < n_kt:
                        emit_S(kt + 1)
                    emit_E(kt)
                    emit_PV(kt - 1)
                emit_PV(n_kt - 1)
                norm_half(1)

            def wo_group(b, g, outT_b):
                """wo partial for outT cols [g*1024,(g+1)*1024)"""
                for o in range(D // 128):
                    for ti, t4 in enumerate((2 * g, 2 * g + 1)):
                        psw = psp.tile([128, 512], f32, tag="mm",
                                        name=f"pw{b}{o}{t4}")
                        nc.tensor.matmul(
                            psw[:],
                            wo[:, o * 128 : (o + 1) * 128],
                            outT_b[:, t4 * 512 : (t4 + 1) * 512],
                            start=True, stop=True,
                        )
                        wout = work.tile([128, 512], bf16, tag="wout")
                        nc.vector.tensor_copy(wout[:], psw[:])
                        dmaq[(o + 2 * ti) % 2].dma_start(
                            out_d[o * 128 : (o + 1) * 128,
                                  b * S + t4 * 512 : b * S + (t4 + 1) * 512],
                            wout[:])

            setup_batch(0)
            outT_b0 = big.tile([128, S], bf16, tag="outT0")
            attention_group(0, 1, 0, outT_b0)
            attention_group(0, 0, 0, outT_b0)
            attention_group(0, 1, 1, outT_b0)
            wo_group(0, 0, outT_b0)
            attention_group(0, 0, 1, outT_b0)
            setup_batch(1)   # fills PE time while batch-0 tail drains
            wo_group(0, 1, outT_b0)
            outT_b1 = big.tile([128, S], bf16, tag="outT1")
            attention_group(1, 1, 0, outT_b1)
            attention_group(1, 0, 0, outT_b1)
            attention_group(1, 1, 1, outT_b1)
            wo_group(1, 0, outT_b1)
            attention_group(1, 0, 1, outT_b1)
            wo_group(1, 1, outT_b1)

    nc.compile()
    return nc


def _host_inputs(x, wq, wk, wv, wo, freqs_cos, freqs_sin):
    """Build the per-core input maps (all host-side transforms are free)."""
    perm = np.concatenate([np.arange(0, HD, 2), np.arange(1, HD, 2)])  # rot-half
    xTf = x.reshape(TOK, D).T.astype(BF16)  # [D, TOK]
    # chunk-contiguous tiling: row block (tb*KT+k) holds xT[k*128:+128, tb*512:+512]
    xT = np.zeros(((TOK // 512) * (D // 128) * 128, 512), BF16)
    for tb in range(TOK // 512):
        for k in range(D // 128):
            r0 = (tb * (D // 128) + k) * 128
            xT[r0 : r0 + 128, :] = xTf[k * 128 : (k + 1) * 128,
                                       tb * 512 : (tb + 1) * 512]

    # signed block-swap P (per 64-dim head): qs_lo = -q_hi, qs_hi = q_lo
    P = np.zeros((HDC, HDC), np.float32)
    for h in range(HPC):
        base = h * HD
        half = HD // 2
        for i in range(half):
            P[base + i, base + half + i] = -1.0
            P[base + half + i, base + i] = 1.0
    PT = np.ascontiguousarray(P.T).astype(BF16)

    # cos/sin expanded to [HDC, TOK]; row j within a head uses freq j%32
    half = HD // 2
    idx = np.concatenate([np.arange(half), np.arange(half)])  # [64]
    cos1 = freqs_cos[:, :].T[idx]  # [64, S]
    sin1 = freqs_sin[:, :].T[idx]
    cosx = np.tile(np.tile(cos1, (HPC, 1)), (1, B)).astype(BF16)  # [128, TOK]
    sinx = np.tile(np.tile(sin1, (HPC, 1)), (1, B)).astype(BF16)

    # additive causal mask for the diagonal tile: 0 where sk<=sq, -1e9 else
    tri = np.where(np.triu(np.ones((128, 128), dtype=bool)), 0.0,
                   -1e9).astype(BF16)
    ident = np.eye(128, dtype=np.float32).astype(BF16)
    ones = np.ones((1, 128), np.float32).astype(BF16)

    in_maps = []
    for c in range(NC):
        rows = []
        for h in range(HPC):
            hg = c * HPC + h
            rows.append(hg * HD + perm)
        rows = np.concatenate(rows)
        wq_c = np.ascontiguousarray(wq[rows, :].T).astype(BF16)  # [D, 128]
        wk_c = np.ascontiguousarray(wk[rows, :].T).astype(BF16)
        vrows = np.arange(c * HDC, (c + 1) * HDC)
        wv_c = np.ascontiguousarray(wv[vrows, :].T).astype(BF16)
        wo_c = np.ascontiguousarray(wo[:, vrows].T).astype(BF16)  # [128, D]
        in_maps.append({
            "xT": xT, "wqT": wq_c, "wkT": wk_c, "wvT": wv_c, "woT": wo_c,
            "PT": PT, "cosx": cosx, "sinx": sinx, "tri": tri,
            "ident": ident, "ones": ones,
        })
    return in_maps


def _install_ntff_hook():
    """Provide antenv.axon_hooks (missing in this image) so that
    run_bass_kernel_spmd(trace=True) can capture an NTFF profile via the
    axon PJRT .so — replicates trn_boot._ntff_profile_via_ctypes."""
    import types, ctypes, contextlib, sys as _sys

    if "antenv.axon_hooks" in _sys.modules:
        return
    so_path = "/opt/axon/libaxon_pjrt.so"
    try:
        lib = ctypes.CDLL(so_path)
    except OSError:
        return
    if not hasattr(lib, "axon_start_nrt_profile"):
        return
    lib.axon_start_nrt_profile.argtypes = [ctypes.POINTER(ctypes.c_int64),
                                           ctypes.c_size_t]
    lib.axon_start_nrt_profile.restype = ctypes.c_int64
    lib.axon_stop_nrt_profile.argtypes = [ctypes.c_char_p]
    lib.axon_stop_nrt_profile.restype = ctypes.c_int64

    @contextlib.contextmanager
    def _hook(output_dir, device_ids):
        import jax
        jax.devices()
        if device_ids:
            ids = (ctypes.c_int64 * len(device_ids))(*device_ids)
            rc = lib.axon_start_nrt_profile(ids, len(device_ids))
        else:
            rc = lib.axon_start_nrt_profile(None, 0)
        if rc != 0:
            raise RuntimeError(f"axon_start_nrt_profile rc={rc}")
        try:
            yield
        finally:
            n = lib.axon_stop_nrt_profile(str(output_dir).encode())
            print(f"ntff profile: {n} file(s) -> {output_dir}", file=sys.stderr)

    mod = types.ModuleType("antenv.axon_hooks")
    mod.get_axon_ntff_profile_hook = lambda: _hook
    mod.set_axon_ntff_profile_hook = lambda h: None
    import antenv
    antenv.axon_hooks = mod
    _sys.modules["antenv.axon_hooks"] = mod


def _is_causal_mask(mask):
    ref = np.where(np.tril(np.ones((S, S), dtype=bool)), 0.0, -1e9)
    return mask.shape == (S, S) and np.array_equal(
        mask.astype(np.float32), ref.astype(np.float32))


def kernel(x, wq, wk, wv, wo, freqs_cos, freqs_sin, mask, _want_trace=False):
    x = np.asarray(x, np.float32)
    mask = np.asarray(mask, np.float32)
    if not _is_causal_mask(mask):
        # general fallback (never hit for the reference's causal mask)
        return _numpy_reference(x, wq, wk, wv, wo, freqs_cos, freqs_sin, mask)

    from concourse.bass_utils import run_bass_kernel_spmd

    if _want_trace:
        _install_ntff_hook()
    if "prog" not in _COMPILED:
        _COMPILED["prog"] = _build_program()
    nc = _COMPILED["prog"]

    in_maps = _host_inputs(np.asarray(x, np.float32), np.asarray(wq, np.float32),
                           np.asarray(wk, np.float32), np.asarray(wv, np.float32),
                           np.asarray(wo, np.float32),
                           np.asarray(freqs_cos, np.float32),
                           np.asarray(freqs_sin, np.float32))
    res = run_bass_kernel_spmd(nc, in_maps, core_ids=list(range(NC)),
                               trace=_want_trace)
    total = np.zeros((D, TOK), np.float32)
    for c in range(NC):
        total += res.results[c]["out"].astype(np.float32)
    out = total.T.reshape(B, S, D).astype(np.float32)
    if _want_trace:
        _COMPILED["last_result"] = res
    return out


def _numpy_reference(x, wq, wk, wv, wo, freqs_cos, freqs_sin, mask):
    import math

    def rope(t):
        t2 = t.reshape(*t.shape[:-1], HD // 2, 2)
        x0, x1 = t2[..., 0], t2[..., 1]
        c = freqs_cos[None, :, None, :]
        s = freqs_sin[None, :, None, :]
        r0 = x0 * c - x1 * s
        r1 = x0 * s + x1 * c
        return np.stack([r0, r1], axis=-1).reshape(t.shape)

    b, s, d = x.shape
    q = (x @ wq.T).reshape(b, s, H, HD)
    k = (x @ wk.T).reshape(b, s, H, HD)
    v = (x @ wv.T).reshape(b, s, H, HD)
    q, k = rope(q), rope(k)
    q = q.transpose(0, 2, 1, 3)
    k = k.transpose(0, 2, 1, 3)
    v = v.transpose(0, 2, 1, 3)
    sc = np.einsum("bhqd,bhkd->bhqk", q, k) / math.sqrt(HD) + mask[None, None]
    sc = sc - sc.max(axis=-1, keepdims=True)
    p = np.exp(sc)
    p /= p.sum(axis=-1, keepdims=True)
    o = np.einsum("bhqk,bhkd->bhqd", p, v).transpose(0, 2, 1, 3).reshape(b, s, d)
    return (o @ wo.T).astype(np.float32)


# revision 15
# speedup vs baseline: 1.2350x; 1.0486x over previous
"""Distributed Trainium2 Bass kernel for causal multi-head attention w/ RoPE.

Problem shapes (hardcoded): B=2, S=2048, D=1024, H=16, HD=64.
Sharding: tensor-parallel over heads — each of 8 cores owns 2 heads
(column slice of wq/wk/wv, row slice of wo). Each core emits its partial
x @ woT contribution; the host sums the 8 partials (the "all-reduce").

Per-core math (all matmuls bf16 on the PE, fp32 PSUM accumulation):
  - q,k,v projections from a host-transposed xT [D, B*S].
  - RoPE: head dims are host-permuted to rotate-half form, so
    rot(q) = q*cos + (P@q)*sin with P a signed block-swap applied by one
    PE matmul; DVE elementwise ops finish the rotation.
  - scores computed transposed: sT[sk, sq] = k_h^T q_h (K=64), causal
    tiles only; exp on ScalarE with the 1/sqrt(HD) scale folded in.
  - the causal mask is applied ON the PE (identity @ maskT accumulated
    into the scores PSUM); one wide exp instruction per sk-tile.
  - the attention inner loop is software-pipelined two sk-tiles deep
    (emit scores of kt+1, exp of kt, PV of kt-1) so the PE never waits
    on the ScalarE exp and keeps its p-state ramp.
  - PV uses v' = [v | 1] so the softmax denominator falls out of the
    matmul as row 64 of the accumulator; normalization = DVE copy
    (frees PSUM early) + fast DVE reciprocal + gpsimd
    partition-broadcast + DVE multiply, off the PE critical path.
  - wo partials are emitted per 1024-column group as soon as that
    group is normalized, so the final-batch tail is short.
"""

import sys

sys.path.insert(0, "/opt/trn_rl_repo")

import numpy as np
import ml_dtypes

B, S, D, H = 2, 2048, 1024, 16
HD = D // H  # 64
NC = 8
HPC = H // NC  # heads per core = 2
HDC = HPC * HD  # head dims per core = 128
TOK = B * S  # 4096
BF16 = ml_dtypes.bfloat16

_COMPILED = {}


def _build_program():
    import concourse.bass as bass
    import concourse.mybir as mybir
    import concourse.bacc as bacc
    from concourse import tile

    f32 = mybir.dt.float32
    bf16 = mybir.dt.bfloat16
    MULT = mybir.AluOpType.mult
    ADD = mybir.AluOpType.add
    EXP = mybir.ActivationFunctionType.Exp

    nc = bacc.Bacc("TRN2", target_bir_lowering=False, debug=False, num_devices=NC)

    xT_d = nc.dram_tensor("xT", [(TOK // 512) * (D // 128) * 128, 512], bf16,
                          kind="ExternalInput").ap()
    wqT_d = nc.dram_tensor("wqT", [D, HDC], bf16, kind="ExternalInput").ap()
    wkT_d = nc.dram_tensor("wkT", [D, HDC], bf16, kind="ExternalInput").ap()
    wvT_d = nc.dram_tensor("wvT", [D, HDC], bf16, kind="ExternalInput").ap()
    woT_d = nc.dram_tensor("woT", [HDC, D], bf16, kind="ExternalInput").ap()
    PT_d = nc.dram_tensor("PT", [HDC, HDC], bf16, kind="ExternalInput").ap()
    cos_d = nc.dram_tensor("cosx", [HDC, TOK], bf16, kind="ExternalInput").ap()
    sin_d = nc.dram_tensor("sinx", [HDC, TOK], bf16, kind="ExternalInput").ap()
    tri_d = nc.dram_tensor("tri", [128, 128], bf16, kind="ExternalInput").ap()
    id_d = nc.dram_tensor("ident", [128, 128], bf16, kind="ExternalInput").ap()
    one_d = nc.dram_tensor("ones", [1, 128], bf16, kind="ExternalInput").ap()
    out_d = nc.dram_tensor("out", [D, TOK], bf16, kind="ExternalOutput").ap()

    KT = D // 128  # 8 contraction tiles for projections
    NTB = TOK // 512  # 8 tok blocks of 512
    NVT = TOK // 128  # 32 tok tiles of 128 (v' tiles)
    VW = HD + 1  # 65: v plus ones column

    with tile.TileContext(nc) as tc:
        with (
            tc.tile_pool(name="big", bufs=1) as big,
            tc.tile_pool(name="work", bufs=3) as work,
            tc.tile_pool(name="etp", bufs=4) as etp,
            tc.tile_pool(name="nrm", bufs=3) as nrm,
            tc.tile_pool(name="scps", bufs=4, space="PSUM") as scps,
            tc.tile_pool(name="psp", bufs=2, space="PSUM") as psp,
            tc.tile_pool(name="pop", bufs=2, space="PSUM") as pop,
        ):
            # ---- DMA order: first-needed tiles first --------------------
            wq = big.tile([128, KT * HDC], bf16, tag="wq")
            wk = big.tile([128, KT * HDC], bf16, tag="wk")
            wv = big.tile([128, KT * HDC], bf16, tag="wv")
            xT = big.tile([128, KT * TOK], bf16, tag="xT")

            dmaq = [nc.sync, nc.gpsimd, nc.scalar]

            def xт_chunk_dma(tb, k, eng):
                r0 = (tb * KT + k) * 128
                eng.dma_start(
                    xT[:, k * TOK + tb * 512 : k * TOK + (tb + 1) * 512],
                    xT_d[r0 : r0 + 128, :])

            # strict priority: weights for q/k/v, then batch-0 x in token
            # order (tb-outer proj consumes ~1MB per 5us), then rope tables,
            # then batch-1 x, then wo.
            for k in range(KT):
                nc.sync.dma_start(wq[:, k * HDC : (k + 1) * HDC],
                                  wqT_d[k * 128 : (k + 1) * 128, :])
                xт_chunk_dma(0, k, dmaq[1 + k % 2])
            for w_sb, w_d in ((wk, wkT_d), (wv, wvT_d)):
                for k in range(KT):
                    nc.sync.dma_start(w_sb[:, k * HDC : (k + 1) * HDC],
                                      w_d[k * 128 : (k + 1) * 128, :])
            for tb in range(1, 4):  # rest of batch 0
                for k in range(KT):
                    xт_chunk_dma(tb, k, dmaq[k % 3])
            PT = big.tile([128, 128], bf16, tag="PT")
            nc.sync.dma_start(PT[:], PT_d[:, :])
            tri = big.tile([128, 128], bf16, tag="tri")
            nc.sync.dma_start(tri[:], tri_d[:, :])
            ident = big.tile([128, 128], bf16, tag="ident")
            nc.sync.dma_start(ident[:], id_d[:, :])
            cosx = big.tile([128, TOK], bf16, tag="cosx")
            sinx = big.tile([128, TOK], bf16, tag="sinx")
            nc.gpsimd.dma_start(cosx[:, 0:S], cos_d[:, 0:S])
            nc.scalar.dma_start(sinx[:, 0:S], sin_d[:, 0:S])
            for tb in range(4, NTB):  # batch 1 (needed ~60us in)
                for k in range(KT):
                    xт_chunk_dma(tb, k, dmaq[k % 3])
            nc.gpsimd.dma_start(cosx[:, S:TOK], cos_d[:, S:TOK])
            nc.scalar.dma_start(sinx[:, S:TOK], sin_d[:, S:TOK])
            wo = big.tile([128, D], bf16, tag="wo")
            nc.sync.dma_start(wo[:], woT_d[:, :])

            q_sb = big.tile([128, TOK], bf16, tag="q")
            k_sb = big.tile([128, TOK], bf16, tag="k")
            v_sb = big.tile([128, TOK], bf16, tag="v")
            rotq = big.tile([128, TOK], bf16, tag="rotq")
            rotk = big.tile([128, TOK], bf16, tag="rotk")
            vp = big.tile([128, NVT * 2 * VW], bf16, tag="vp")
            oh1 = big.tile([64, S], bf16, tag="oh1")

            def vp_head(kt, h):
                base = kt * 2 * VW + h * VW
                return vp[:, base : base + VW]

            def setup_batch(b):
                """projections + RoPE + v' tiles for batch b's 2048 cols"""
                for tb in range(b * 4, b * 4 + 4):
                    for dst, w_sb in ((q_sb, wq), (k_sb, wk), (v_sb, wv)):
                        ps = psp.tile([128, 512], f32, tag="mm",
                                      name=f"ps{b}{tb}")
                        for k in range(KT):
                            nc.tensor.matmul(
                                ps[:],
                                w_sb[:, k * HDC : (k + 1) * HDC],
                                xT[:, k * TOK + tb * 512 : k * TOK + (tb + 1) * 512],
                                start=(k == 0), stop=(k == KT - 1),
                            )
                        if b == 0:
                            nc.scalar.copy(dst[:, tb * 512 : (tb + 1) * 512],
                                           ps[:])
                        else:
                            nc.vector.tensor_copy(
                                dst[:, tb * 512 : (tb + 1) * 512], ps[:])
                for src, dst in ((q_sb, rotq), (k_sb, rotk)):
                    for tb in range(b * 4, b * 4 + 4):
                        blk = slice(tb * 512, (tb + 1) * 512)
                        pss = psp.tile([128, 512], f32, tag="mm",
                                        name=f"pr{b}{tb}")
                        nc.tensor.matmul(pss[:], PT[:], src[:, blk],
                                         start=True, stop=True)
                        t1 = work.tile([128, 512], bf16, tag="ropet1")
                        nc.vector.tensor_tensor(t1[:], src[:, blk],
                                                cosx[:, blk], MULT)
                        t2 = work.tile([128, 512], bf16, tag="ropet2")
                        nc.vector.tensor_tensor(t2[:], pss[:], sinx[:, blk], MULT)
                        nc.vector.tensor_tensor(dst[:, blk], t1[:], t2[:], ADD)
                for kt in range(b * 16, b * 16 + 16):
                    pst = psp.tile([128, 128], bf16, tag="mm",
                                    name=f"pv{kt}")
                    nc.tensor.transpose(pst[:],
                                        v_sb[:, kt * 128 : (kt + 1) * 128],
                                        ident[:])
                    for h in range(HPC):
                        base = kt * 2 * VW + h * VW
                        nc.vector.tensor_copy(vp[:, base : base + HD],
                                              pst[:, h * HD : (h + 1) * HD])
                        nc.gpsimd.memset(vp[:, base + HD : base + VW], 1.0)

            def attention_group(b, h, g, outT_b):
                """scores+softmax+PV for head h, sq cols [g*1024,(g+1)*1024)

                Two-deep software pipeline: iteration kt emits scores of
                kt+1, the exp of kt, and PV of kt-1, so the PE is never
                blocked on the ScalarE exp."""
                hsl = slice(h * HD, (h + 1) * HD)
                g0 = 1024 * g
                n_kt = 8 * g + 8
                kt_max = [(g0 + 512 * (i + 1)) // 128 - 1 for i in range(2)]
                po = [pop.tile([VW, 512], f32, tag="po",
                               name=f"po{b}{h}{g}{i}") for i in range(2)]
                sc = {}
                et = {}

                def cols(kt, i):
                    w0 = 128 * kt
                    s0 = max(w0, g0 + 512 * i)
                    e0 = g0 + 512 * (i + 1)
                    return (s0, e0) if s0 < e0 else None

                def emit_S(kt):
                    w0 = 128 * kt
                    diag_i = (w0 - g0) // 512 if w0 >= g0 else -1
                    for i in range(2):
                        ci = cols(kt, i)
                        if ci is None:
                            continue
                        s0, e0 = ci
                        h0c = g0 + 512 * i
                        t = scps.tile([128, 512], f32, tag="sc",
                                      name=f"sc{b}{h}{g}{kt}{i}")
                        sc[(kt, i)] = t
                        nc.tensor.matmul(
                            t[:, s0 - h0c : e0 - h0c],
                            rotk[hsl, b * S + w0 : b * S + w0 + 128],
                            rotq[hsl, b * S + s0 : b * S + e0],
                            start=True, stop=(i != diag_i),
                        )
                        if i == diag_i:  # diag tile: accumulate -1e9 mask
                            nc.tensor.matmul(
                                t[:, w0 - h0c : w0 - h0c + 128],
                                ident[:], tri[:],
                                start=False, stop=True,
                            )

                def emit_E(kt):
                    for i in range(2):
                        ci = cols(kt, i)
                        if ci is None:
                            continue
                        s0, e0 = ci
                        h0c = g0 + 512 * i
                        tt = etp.tile([128, 512], bf16, tag="expT",
                                      name=f"et{b}{h}{g}{kt}{i}")
                        et[(kt, i)] = tt
                        nc.scalar.activation(
                            tt[:, s0 - h0c : e0 - h0c],
                            sc.pop((kt, i))[:, s0 - h0c : e0 - h0c],
                            EXP, scale=0.125)

                def emit_PV(kt):
                    for i in range(2):
                        ci = cols(kt, i)
                        if ci is None:
                            continue
                        s0, e0 = ci
                        h0c = g0 + 512 * i
                        nc.tensor.matmul(
                            po[i][:, s0 - h0c : e0 - h0c],
                            vp_head(b * (NVT // B) + kt, h),
                            et.pop((kt, i))[:, s0 - h0c : e0 - h0c],
                            start=(kt == 0), stop=(kt == kt_max[i]),
                        )
                    if kt == kt_max[0]:
                        norm_half(0)

                def norm_half(i):
                    c0 = g0 + 512 * i
                    d0 = nrm.tile([1, 512], f32, tag="d0")
                    nc.vector.tensor_copy(d0[0:1, :], po[i][HD : HD + 1, :])
                    poc = nrm.tile([VW, 512], f32, tag="poc")
                    nc.vector.tensor_copy(poc[:], po[i][:])  # frees PSUM slot
                    db = nrm.tile([64, 512], f32, tag="db")
                    nc.gpsimd.partition_broadcast(db[:, :], d0[0:1, :])
                    rb_sb = nrm.tile([64, 512], f32, tag="rb")
                    nc.vector.reciprocal_approx_fast(rb_sb[:, :], db[:, :])
                    ocols = slice(c0, c0 + 512)
                    if h == 0:
                        nc.vector.tensor_tensor(outT_b[0:HD, ocols],
                                                poc[0:HD, :], rb_sb[:, :], MULT)
                    else:
                        nc.vector.tensor_tensor(oh1[:, ocols], poc[0:HD, :],
                                                rb_sb[:, :], MULT)
                        nc.sync.dma_start(outT_b[HD : 2 * HD, ocols],
                                          oh1[:, ocols])

                emit_S(0)
                if n_kt > 1:
                    emit_S(1)
                emit_E(0)
                for kt in range(1, n_kt):
                    if kt + 1 < n_kt:
                        emit_S(kt + 1)
                    emit_E(kt)
                    emit_PV(kt - 1)
                emit_PV(n_kt - 1)
                norm_half(1)

            def wo_group(b, t4, outT_b):
                """wo partial for outT cols [t4*512,(t4+1)*512)"""
                for o in range(D // 128):
                    psw = psp.tile([128, 512], f32, tag="mm",
                                   name=f"pw{b}{o}{t4}")
                    nc.tensor.matmul(
                        psw[:],
                        wo[:, o * 128 : (o + 1) * 128],
                        outT_b[:, t4 * 512 : (t4 + 1) * 512],
                        start=True, stop=True,
                    )
                    wout = work.tile([128, 512], bf16, tag="wout")
                    nc.vector.tensor_copy(wout[:], psw[:])
                    dmaq[o % 2].dma_start(
                        out_d[o * 128 : (o + 1) * 128,
                              b * S + t4 * 512 : b * S + (t4 + 1) * 512],
                        wout[:])

            setup_batch(0)
            outT_b0 = big.tile([128, S], bf16, tag="outT0")
            attention_group(0, 1, 0, outT_b0)
            attention_group(0, 0, 0, outT_b0)
            attention_group(0, 1, 1, outT_b0)
            wo_group(0, 0, outT_b0)
            wo_group(0, 1, outT_b0)
            attention_group(0, 0, 1, outT_b0)
            setup_batch(1)   # fills PE time while batch-0 tail drains
            wo_group(0, 2, outT_b0)
            wo_group(0, 3, outT_b0)
            outT_b1 = big.tile([128, S], bf16, tag="outT1")
            attention_group(1, 1, 0, outT_b1)
            attention_group(1, 0, 0, outT_b1)
            attention_group(1, 1, 1, outT_b1)
            wo_group(1, 0, outT_b1)
            wo_group(1, 1, outT_b1)
            attention_group(1, 0, 1, outT_b1)
            wo_group(1, 2, outT_b1)
            wo_group(1, 3, outT_b1)

    nc.compile()
    return nc


def _host_inputs(x, wq, wk, wv, wo, freqs_cos, freqs_sin):
    """Build the per-core input maps (all host-side transforms are free)."""
    perm = np.concatenate([np.arange(0, HD, 2), np.arange(1, HD, 2)])  # rot-half
    xTf = x.reshape(TOK, D).T.astype(BF16)  # [D, TOK]
    # chunk-contiguous tiling: row block (tb*KT+k) holds xT[k*128:+128, tb*512:+512]
    xT = np.zeros(((TOK // 512) * (D // 128) * 128, 512), BF16)
    for tb in range(TOK // 512):
        for k in range(D // 128):
            r0 = (tb * (D // 128) + k) * 128
            xT[r0 : r0 + 128, :] = xTf[k * 128 : (k + 1) * 128,
                                       tb * 512 : (tb + 1) * 512]

    # signed block-swap P (per 64-dim head): qs_lo = -q_hi, qs_hi = q_lo
    P = np.zeros((HDC, HDC), np.float32)
    for h in range(HPC):
        base = h * HD
        half = HD // 2
        for i in range(half):
            P[base + i, base + half + i] = -1.0
            P[base + half + i, base + i] = 1.0
    PT = np.ascontiguousarray(P.T).astype(BF16)

    # cos/sin expanded to [HDC, TOK]; row j within a head uses freq j%32
    half = HD // 2
    idx = np.concatenate([np.arange(half), np.arange(half)])  # [64]
    cos1 = freqs_cos[:, :].T[idx]  # [64, S]
    sin1 = freqs_sin[:, :].T[idx]
    cosx = np.tile(np.tile(cos1, (HPC, 1)), (1, B)).astype(BF16)  # [128, TOK]
    sinx = np.tile(np.tile(sin1, (HPC, 1)), (1, B)).astype(BF16)

    # additive causal mask for the diagonal tile: 0 where sk<=sq, -1e9 else
    tri = np.where(np.triu(np.ones((128, 128), dtype=bool)), 0.0,
                   -1e9).astype(BF16)
    ident = np.eye(128, dtype=np.float32).astype(BF16)
    ones = np.ones((1, 128), np.float32).astype(BF16)

    in_maps = []
    for c in range(NC):
        rows = []
        for h in range(HPC):
            hg = c * HPC + h
            rows.append(hg * HD + perm)
        rows = np.concatenate(rows)
        wq_c = np.ascontiguousarray(wq[rows, :].T).astype(BF16)  # [D, 128]
        wk_c = np.ascontiguousarray(wk[rows, :].T).astype(BF16)
        vrows = np.arange(c * HDC, (c + 1) * HDC)
        wv_c = np.ascontiguousarray(wv[vrows, :].T).astype(BF16)
        wo_c = np.ascontiguousarray(wo[:, vrows].T).astype(BF16)  # [128, D]
        in_maps.append({
            "xT": xT, "wqT": wq_c, "wkT": wk_c, "wvT": wv_c, "woT": wo_c,
            "PT": PT, "cosx": cosx, "sinx": sinx, "tri": tri,
            "ident": ident, "ones": ones,
        })
    return in_maps


def _install_ntff_hook():
    """Provide antenv.axon_hooks (missing in this image) so that
    run_bass_kernel_spmd(trace=True) can capture an NTFF profile via the
    axon PJRT .so — replicates trn_boot._ntff_profile_via_ctypes."""
    import types, ctypes, contextlib, sys as _sys

    if "antenv.axon_hooks" in _sys.modules:
        return
    so_path = "/opt/axon/libaxon_pjrt.so"
    try:
        lib = ctypes.CDLL(so_path)
    except OSError:
        return
    if not hasattr(lib, "axon_start_nrt_profile"):
        return
    lib.axon_start_nrt_profile.argtypes = [ctypes.POINTER(ctypes.c_int64),
                                           ctypes.c_size_t]
    lib.axon_start_nrt_profile.restype = ctypes.c_int64
    lib.axon_stop_nrt_profile.argtypes = [ctypes.c_char_p]
    lib.axon_stop_nrt_profile.restype = ctypes.c_int64

    @contextlib.contextmanager
    def _hook(output_dir, device_ids):
        import jax
        jax.devices()
        if device_ids:
            ids = (ctypes.c_int64 * len(device_ids))(*device_ids)
            rc = lib.axon_start_nrt_profile(ids, len(device_ids))
        else:
            rc = lib.axon_start_nrt_profile(None, 0)
        if rc != 0:
            raise RuntimeError(f"axon_start_nrt_profile rc={rc}")
        try:
            yield
        finally:
            n = lib.axon_stop_nrt_profile(str(output_dir).encode())
            print(f"ntff profile: {n} file(s) -> {output_dir}", file=sys.stderr)

    mod = types.ModuleType("antenv.axon_hooks")
    mod.get_axon_ntff_profile_hook = lambda: _hook
    mod.set_axon_ntff_profile_hook = lambda h: None
    import antenv
    antenv.axon_hooks = mod
    _sys.modules["antenv.axon_hooks"] = mod


def _is_causal_mask(mask):
    ref = np.where(np.tril(np.ones((S, S), dtype=bool)), 0.0, -1e9)
    return mask.shape == (S, S) and np.array_equal(
        mask.astype(np.float32), ref.astype(np.float32))


def kernel(x, wq, wk, wv, wo, freqs_cos, freqs_sin, mask, _want_trace=False):
    x = np.asarray(x, np.float32)
    mask = np.asarray(mask, np.float32)
    if not _is_causal_mask(mask):
        # general fallback (never hit for the reference's causal mask)
        return _numpy_reference(x, wq, wk, wv, wo, freqs_cos, freqs_sin, mask)

    from concourse.bass_utils import run_bass_kernel_spmd

    if _want_trace:
        _install_ntff_hook()
    if "prog" not in _COMPILED:
        _COMPILED["prog"] = _build_program()
    nc = _COMPILED["prog"]

    in_maps = _host_inputs(np.asarray(x, np.float32), np.asarray(wq, np.float32),
                           np.asarray(wk, np.float32), np.asarray(wv, np.float32),
                           np.asarray(wo, np.float32),
                           np.asarray(freqs_cos, np.float32),
                           np.asarray(freqs_sin, np.float32))
    res = run_bass_kernel_spmd(nc, in_maps, core_ids=list(range(NC)),
                               trace=_want_trace)
    total = np.zeros((D, TOK), np.float32)
    for c in range(NC):
        total += res.results[c]["out"].astype(np.float32)
    out = total.T.reshape(B, S, D).astype(np.float32)
    if _want_trace:
        _COMPILED["last_result"] = res
    return out


def _numpy_reference(x, wq, wk, wv, wo, freqs_cos, freqs_sin, mask):
    import math

    def rope(t):
        t2 = t.reshape(*t.shape[:-1], HD // 2, 2)
        x0, x1 = t2[..., 0], t2[..., 1]
        c = freqs_cos[None, :, None, :]
        s = freqs_sin[None, :, None, :]
        r0 = x0 * c - x1 * s
        r1 = x0 * s + x1 * c
        return np.stack([r0, r1], axis=-1).reshape(t.shape)

    b, s, d = x.shape
    q = (x @ wq.T).reshape(b, s, H, HD)
    k = (x @ wk.T).reshape(b, s, H, HD)
    v = (x @ wv.T).reshape(b, s, H, HD)
    q, k = rope(q), rope(k)
    q = q.transpose(0, 2, 1, 3)
    k = k.transpose(0, 2, 1, 3)
    v = v.transpose(0, 2, 1, 3)
    sc = np.einsum("bhqd,bhkd->bhqk", q, k) / math.sqrt(HD) + mask[None, None]
    sc = sc - sc.max(axis=-1, keepdims=True)
    p = np.exp(sc)
    p /= p.sum(axis=-1, keepdims=True)
    o = np.einsum("bhqk,bhkd->bhqd", p, v).transpose(0, 2, 1, 3).reshape(b, s, d)
    return (o @ wo.T).astype(np.float32)
